# revision 43
# baseline (speedup 1.0000x reference)
"""Trainium2 Bass kernel for multi-head attention (B=4, N=4096, D=384, H=6).

Sharding: 8 cores = 4 batches x 2 head-groups (3 heads each).

Design notes:
- Host pre-transposes x (bf16) and weights (wT/wpT bf16), folding the 1/sqrt(dh)
  scale into the q rows of W_qkv. No PE transposes on device.
- W column order per group: [q_h0 q_h1 | k_h0 k_h1 | q_h2 k_h2 | v_h0 v_h1 v_h2]
  so qkv matmuls produce M=128-packed outputs with lane-aligned copies.
- Heads 0/1 are row-packed BY HEAD in the score matmuls (no q/k replication);
  head 2 is row-packed across key-tile pairs with DMA-replicated q2/k2 halves.
- exp(scores) is split across ScalarE (true exp ACT) and VectorE (Schraudolph:
  i16 = s*128/ln2 + B computed by one tensor_scalar, bitcast as bf16) to break
  the ACT-engine bottleneck (~440us of the 660us baseline).
- Softmax denominator comes free from a ones-column appended to V (PSUM row 64
  of the PV accumulator); 1/S via reciprocal_approx_fast (the exact DVE
  reciprocal at 6 cyc/elem cost 80us + long waits in the baseline).
"""

import math
import os
import sys

import numpy as np

B, NSEQ, D = 4, 4096, 384
H, DH = 6, 64
HPC = 3  # heads per core
NCORES = 8
GCOLS = HPC * DH  # 192
SCALE = float(DH) ** -0.5

# Schraudolph exp in bf16-bitpattern domain: i16 = round(s*128/ln2 + B)
EXP_M = 128.0 / math.log(2.0)
EXP_B = 128.0 * 127.0 - 128.0 * (math.log2(1.0861) / 2.0) + 0.5

# Fraction of exp tiles handled by ScalarE (rest go to VectorE Schraudolph)
ACT_FRAC = 0.58
PV_DELAY = 14  # PV matmul entries the queue trails by (2 per score pair)


def _ensure_path():
    p = "/opt/trn_rl_repo"
    if os.path.isdir(p) and p not in sys.path:
        sys.path.insert(0, p)


def build_nc(n_seq=NSEQ):
    """Build the single-core Bass program (SPMD across 8 cores)."""
    _ensure_path()
    from contextlib import ExitStack

    import concourse.bacc as bacc
    import concourse.tile as tile
    from concourse import mybir

    f32 = mybir.dt.float32
    bf16 = mybir.dt.bfloat16
    i16 = mybir.dt.int16
    EXP = mybir.ActivationFunctionType.Exp
    MULT = mybir.AluOpType.mult
    ADD = mybir.AluOpType.add

    assert n_seq % 512 == 0
    nt = n_seq // 128
    nch = n_seq // 512

    nc = bacc.Bacc("TRN2", target_bir_lowering=False, debug=False)

    xT_d = nc.dram_tensor("xT", [D, n_seq], bf16, kind="ExternalInput")
    wT_d = nc.dram_tensor("wT", [128, 3 * 576], bf16, kind="ExternalInput")
    wpT_d = nc.dram_tensor("wpT", [64, 3 * D], bf16, kind="ExternalInput")
    y_d = nc.dram_tensor("yT", [D, n_seq], f32, kind="ExternalOutput")

    dbg = os.environ.get("KERNEL_DEBUG", "") == "1"
    if dbg:
        dbg_qk_d = nc.dram_tensor("dbg_qk", [128, 4, n_seq], bf16, kind="ExternalOutput")
        dbg_v_d = nc.dram_tensor("dbg_v", [128, (n_seq // 128) * 3 * 65], bf16, kind="ExternalOutput")
        dbg_e_d = nc.dram_tensor("dbg_e", [128, 4, 2, 512], bf16, kind="ExternalOutput")
        dbg_h_d = nc.dram_tensor("dbg_h", [64, 3, 512], bf16, kind="ExternalOutput")

    with tile.TileContext(nc) as tc, ExitStack() as ctx:
        constp = ctx.enter_context(tc.tile_pool(name="const", bufs=1))
        persist = ctx.enter_context(tc.tile_pool(name="persist", bufs=1))
        hpool = ctx.enter_context(tc.tile_pool(name="headsp", bufs=2))
        ypool = ctx.enter_context(tc.tile_pool(name="youtp", bufs=3))
        epool = ctx.enter_context(tc.tile_pool(name="epool", bufs=11))
        rcpool = ctx.enter_context(tc.tile_pool(name="rcp", bufs=2))

        ones = constp.tile([65, 64], f32)
        nc.vector.memset(ones, 1.0)

        # Transposed weights (host-prepped): wT[p, d3, col] = W_sel[col, 128*d3+p]
        wT = persist.tile([128, 3, 576], bf16)
        wpT = persist.tile([64, 3, D], bf16)
        # xT[p, d3, n] = x[n, 128*d3 + p] (host-prepped, bf16)
        xT = persist.tile([128, 3, n_seq], bf16)
        # q/k transposed layouts [c, n]:
        #   tq01/tk01: partitions 0:64 = head0, 64:128 = head1 (no replication)
        #   tq2/tk2: head2 replicated across both halves (via DMA)
        tq01 = persist.tile([128, n_seq], bf16)
        tk01 = persist.tile([128, n_seq], bf16)
        tq2 = persist.tile([128, n_seq], bf16)
        tk2 = persist.tile([128, n_seq], bf16)
        # V natural layout per key-tile, per head, ones column at col 64
        v_sb = persist.tile([128, nt, 3, 65], bf16)
        ones_nt = constp.tile([128, nt * 3], f32)
        nc.vector.memset(ones_nt, 1.0)
        nc.vector.tensor_copy(
            v_sb[:, :, :, 64:65].rearrange("p t h o -> p (t h o)"), ones_nt
        )

        nc.sync.dma_start(out=wT.rearrange("p a b -> p (a b)"), in_=wT_d[:, :])
        nc.sync.dma_start(out=wpT.rearrange("p a b -> p (a b)"), in_=wpT_d[:, :])
        # x loads chunk-major so qkv chunk 0 can start early
        for j in range(nch):
            for d3 in range(3):
                nc.sync.dma_start(
                    out=xT[:, d3, 512 * j : 512 * (j + 1)],
                    in_=xT_d[128 * d3 : 128 * (d3 + 1), 512 * j : 512 * (j + 1)],
                )

        # ---- qkv ----
        with (
            tc.tile_pool(name="pqk", bufs=4, space="PSUM") as pqk,
            tc.tile_pool(name="pvp", bufs=3, space="PSUM") as pvp,
        ):
            for j in range(nch):
                jsl = slice(512 * j, 512 * (j + 1))
                for c0, copies in (
                    (0, "q01"),
                    (128, "k01"),
                    (256, "q2k2"),
                ):
                    ps = pqk.tile([128, 512], f32, tag="pqk")
                    for d3 in range(3):
                        nc.tensor.matmul(
                            ps,
                            wT[:, d3, c0 : c0 + 128],
                            xT[:, d3, jsl],
                            start=(d3 == 0),
                            stop=(d3 == 2),
                        )
                    if copies == "q01":
                        nc.vector.tensor_copy(tq01[:, jsl], ps)
                    elif copies == "k01":
                        nc.vector.tensor_copy(tk01[:, jsl], ps)
                    else:
                        nc.scalar.copy(tq2[0:64, jsl], ps[0:64, :])
                        nc.scalar.copy(tk2[64:128, jsl], ps[64:128, :])
                for tt in range(4 * j, 4 * j + 4):
                    psv = pvp.tile([128, 192], f32, tag="pv")
                    for d3 in range(3):
                        nc.tensor.matmul(
                            psv,
                            xT[:, d3, 128 * tt : 128 * (tt + 1)],
                            wT[:, d3, 384:576],
                            start=(d3 == 0),
                            stop=(d3 == 2),
                        )
                    nc.scalar.copy(
                        v_sb[:, tt, :, 0:64],
                        psv[:, :].rearrange("p (h c) -> p h c", h=3),
                    )

        # replicate head2 q/k across the other partition half
        nc.sync.dma_start(out=tq2[64:128, :], in_=tq2[0:64, :])
        nc.sync.dma_start(out=tk2[0:64, :], in_=tk2[64:128, :])

        if dbg:
            for i, tqk in enumerate((tq01, tk01, tq2, tk2)):
                nc.sync.dma_start(out=dbg_qk_d[:, i, :], in_=tqk)
            nc.sync.dma_start(
                out=dbg_v_d[:, :], in_=v_sb.rearrange("p t h o -> p (t h o)")
            )

        # ---- attention + projection ----
        with (
            tc.tile_pool(name="psc", bufs=3, space="PSUM") as psc,
            tc.tile_pool(name="pop", bufs=2, space="PSUM") as pop,
        ):
            exp_state = [0.0]

            def exp_op(e_t, ps):
                # per-bank halves: each half's exp depends only on its own
                # score matmul and frees its PSUM bank independently
                exp_state[0] += ACT_FRAC
                if exp_state[0] >= 1.0:
                    exp_state[0] -= 1.0
                    for s in range(2):
                        nc.scalar.activation(e_t[:, s, :], ps[:, s, :], EXP)
                else:
                    for s in range(2):
                        nc.vector.tensor_scalar(
                            out=e_t[:, s, :].bitcast(i16),
                            in0=ps[:, s, :],
                            scalar1=EXP_M,
                            scalar2=EXP_B,
                            op0=MULT,
                            op1=ADD,
                        )

            pv_queue = []
            stops_emitted = set()  # tokens whose stop-matmul has been traced
            token_ctr = [0]

            def new_token():
                token_ctr[0] += 1
                return token_ctr[0]

            def flush_pv():
                po, vap, eap, st, sp = pv_queue.pop(0)
                nc.tensor.matmul(po, vap, eap, start=st, stop=(sp is not None))
                if sp is not None:
                    stops_emitted.add(sp)

            def pop_tail(pending):
                """Emit the next tail part iff its po's stop-MM is traced."""
                if not pending:
                    return
                po_id, part = pending[0]
                if po_id in stops_emitted:
                    pending.pop(0)
                    part()

            def drain_tail(pending):
                while pending:
                    po_id, part = pending[0]
                    while po_id not in stops_emitted:
                        flush_pv()
                    pending.pop(0)
                    part()

            def make_tail(j, h, po_t, tok, headsT):
                # The tail is split so (a) the PSUM accumulator po_t is freed
                # by its FIRST part (one ScalarE copy to SBUF) — the next
                # chunk's PV matmuls WAR-wait on that bank at the PE FIFO
                # head otherwise — and (b) the broadcast matmul is emitted a
                # few iterations later, after the reciprocal chain has
                # cleared the busy ACT/DVE queues.
                # normalize is split into two parts popped on consecutive
                # iterations so its engine ops spread across the ACT/DVE
                # queues instead of landing ahead of the exp ops that free
                # score-PSUM banks (the chunk-boundary PE stall).
                state = {}

                def norm_a():
                    rc = rcpool.tile([65, 2, 512], f32, tag="rc")
                    state["rc"] = rc
                    # reciprocal_approx_fast requires base partition 0 — copy
                    # the whole tile (denominator row 64 included) to SBUF and
                    # run the approx over all 65 rows (cost is free-dim-bound)
                    nc.vector.tensor_copy(rc[:, 0, :], po_t[:, :])
                    nc.vector.reciprocal_approx_fast(rc[:, 1, :], rc[:, 0, :])

                def norm_b():
                    rc = state["rc"]
                    pb = psc.tile([64, 512], f32, tag="ps")
                    nc.tensor.matmul(pb, ones[64:65, :], rc[64:65, 1, :])
                    rb = rcpool.tile([64, 512], f32, tag="rb")
                    nc.vector.tensor_copy(rb, pb)
                    nc.vector.tensor_mul(headsT[:, h, :], po_t[0:64, :], rb)

                def proj(e3):
                    def f():
                        pp = psc.tile([128, 512], f32, tag="ps")
                        for hh in range(3):
                            nc.tensor.matmul(
                                pp,
                                wpT[:, hh, 128 * e3 : 128 * (e3 + 1)],
                                headsT[:, hh, :],
                                start=(hh == 0),
                                stop=(hh == 2),
                            )
                        y_sb = ypool.tile([128, 512], f32, tag="y_sb")
                        nc.vector.tensor_copy(y_sb, pp)
                        nc.sync.dma_start(
                            out=y_d[
                                128 * e3 : 128 * (e3 + 1), 512 * j : 512 * (j + 1)
                            ],
                            in_=y_sb,
                        )

                    return f

                parts = [(tok, norm_a), (tok, norm_b)]
                if h == 2:
                    parts += [(tok, proj(e3)) for e3 in range(3)]
                return parts

            pending_tail = []
            for j in range(nch):
                jsl = slice(512 * j, 512 * (j + 1))
                headsT = hpool.tile([64, 3, 512], bf16, tag="headsT")

                # phase A: heads 0/1 row-packed by head, one key-tile per iter
                po0 = pop.tile([65, 512], f32, tag="po")
                po1 = pop.tile([65, 512], f32, tag="po")
                tok0, tok1 = new_token(), new_token()
                for t in range(nt):
                    tsl = slice(128 * t, 128 * (t + 1))
                    ps = psc.tile([128, 2, 512], f32, tag="ps")
                    nc.tensor.matmul(ps[:, 0, :], tk01[0:64, tsl], tq01[0:64, jsl])
                    nc.tensor.matmul(
                        ps[:, 1, :], tk01[64:128, tsl], tq01[64:128, jsl]
                    )
                    e_t = epool.tile([128, 2, 512], bf16, tag="e_t")
                    exp_op(e_t, ps)
                    if dbg and j == 0 and t < 4:
                        nc.sync.dma_start(out=dbg_e_d[:, t, :, :], in_=e_t)
                    pv_queue.append(
                        (
                            po0,
                            v_sb[:, t, 0, :],
                            e_t[:, 0, :],
                            t == 0,
                            tok0 if t == nt - 1 else None,
                        )
                    )
                    pv_queue.append(
                        (
                            po1,
                            v_sb[:, t, 1, :],
                            e_t[:, 1, :],
                            t == 0,
                            tok1 if t == nt - 1 else None,
                        )
                    )
                    while len(pv_queue) > PV_DELAY:
                        flush_pv()
                    pop_tail(pending_tail)
                drain_tail(pending_tail)
                pending_tail = make_tail(j, 0, po0, tok0, headsT) + make_tail(
                    j, 1, po1, tok1, headsT
                )

                # phase B: head 2 row-packed across key-tile pairs
                po2 = pop.tile([65, 512], f32, tag="po")
                tok2 = new_token()
                for tp in range(nt // 2):
                    t0 = 2 * tp
                    ps = psc.tile([128, 2, 512], f32, tag="ps")
                    nc.tensor.matmul(
                        ps[:, 0, :],
                        tk2[0:64, 128 * t0 : 128 * (t0 + 1)],
                        tq2[0:64, jsl],
                    )
                    nc.tensor.matmul(
                        ps[:, 1, :],
                        tk2[64:128, 128 * (t0 + 1) : 128 * (t0 + 2)],
                        tq2[64:128, jsl],
                    )
                    e_t = epool.tile([128, 2, 512], bf16, tag="e_t")
                    exp_op(e_t, ps)
                    pv_queue.append(
                        (po2, v_sb[:, t0, 2, :], e_t[:, 0, :], t0 == 0, None)
                    )
                    pv_queue.append(
                        (
                            po2,
                            v_sb[:, t0 + 1, 2, :],
                            e_t[:, 1, :],
                            False,
                            tok2 if t0 + 1 == nt - 1 else None,
                        )
                    )
                    while len(pv_queue) > PV_DELAY:
                        flush_pv()
                    pop_tail(pending_tail)
                drain_tail(pending_tail)
                pending_tail = make_tail(j, 2, po2, tok2, headsT)

            while pv_queue:
                flush_pv()
            drain_tail(pending_tail)
            if dbg:
                # headsT of the LAST chunk (hpool tag rotation: last allocated)
                nc.sync.dma_start(out=dbg_h_d[:, :, :], in_=headsT)

    nc.compile()
    return nc


def shard_inputs(x, W_qkv, W_proj):
    """Full inputs -> per-core input maps (host-side transpose + pack)."""
    import ml_dtypes

    bf16 = ml_dtypes.bfloat16
    x = np.asarray(x, dtype=np.float32)
    W_qkv = np.asarray(W_qkv, dtype=np.float32)
    W_proj = np.asarray(W_proj, dtype=np.float32)
    d = W_qkv.shape[1]
    in_maps = []
    for c in range(NCORES):
        b, g = divmod(c, 2)
        sl = slice(GCOLS * g, GCOLS * (g + 1))
        q = W_qkv[0 * d :][sl] * SCALE  # [192, 384] (scale folded into q)
        k = W_qkv[1 * d :][sl]
        v = W_qkv[2 * d :][sl]
        w_sel = np.concatenate(
            [q[0:64], q[64:128], k[0:64], k[64:128], q[128:192], k[128:192], v],
            axis=0,
        )  # [576, 384]
        # wT[p, d3, col] = w_sel[col, 128*d3 + p]
        wT = np.ascontiguousarray(
            w_sel.T.reshape(3, 128, 576).transpose(1, 0, 2).reshape(128, 3 * 576)
        ).astype(bf16)
        wp_g = W_proj[:, sl]  # [384, 192]
        # wpT[p, h, e] = wp_g[e, 64h + p]
        wpT = np.ascontiguousarray(
            wp_g.T.reshape(3, 64, d).transpose(1, 0, 2).reshape(64, 3 * d)
        ).astype(bf16)
        xT = np.ascontiguousarray(x[b].T).astype(bf16)
        in_maps.append({"xT": xT, "wT": wT, "wpT": wpT})
    return in_maps


def combine_outputs(yTs, b_proj, n_seq=NSEQ):
    """Per-core yT partials -> full [B, N, D] output."""
    b_proj = np.asarray(b_proj, dtype=np.float32)
    y = np.empty((B, n_seq, D), dtype=np.float32)
    for b in range(B):
        y[b] = (yTs[2 * b] + yTs[2 * b + 1]).T + b_proj
    return y


_NC_CACHE = {}


def kernel(**inputs):
    _ensure_path()
    from concourse.bass_utils import run_bass_kernel_spmd

    x = np.asarray(inputs["x"], dtype=np.float32)
    W_qkv = np.asarray(inputs["W_qkv"], dtype=np.float32)
    W_proj = np.asarray(inputs["W_proj"], dtype=np.float32)
    b_proj = np.asarray(inputs["b_proj"], dtype=np.float32)

    n_seq = x.shape[1]
    if n_seq not in _NC_CACHE:
        _NC_CACHE[n_seq] = build_nc(n_seq)
    nc = _NC_CACHE[n_seq]

    in_maps = shard_inputs(x, W_qkv, W_proj)
    res = run_bass_kernel_spmd(nc, in_maps, core_ids=list(range(NCORES)))
    yTs = [r["yT"] for r in res.results]
    return combine_outputs(yTs, b_proj, n_seq)


if __name__ == "__main__":
    rng = np.random.default_rng(0)
    n = 512
    x = rng.standard_normal((B, n, D), dtype=np.float32)
    wq = (rng.standard_normal((3 * D, D), dtype=np.float32) / np.sqrt(D)).astype(
        np.float32
    )
    wp = (rng.standard_normal((D, D), dtype=np.float32) / np.sqrt(D)).astype(
        np.float32
    )
    bp = np.zeros(D, np.float32)
    out = kernel(x=x, W_qkv=wq, W_proj=wp, b_proj=bp)
    print(out.shape, out.dtype)


# revision 44
# speedup vs baseline: 1.0059x; 1.0059x over previous
"""Trainium2 Bass kernel for multi-head attention (B=4, N=4096, D=384, H=6).

Sharding: 8 cores = 4 batches x 2 head-groups (3 heads each).

Design notes:
- Host pre-transposes x (bf16) and weights (wT/wpT bf16), folding the 1/sqrt(dh)
  scale into the q rows of W_qkv. No PE transposes on device.
- W column order per group: [q_h0 q_h1 | k_h0 k_h1 | q_h2 k_h2 | v_h0 v_h1 v_h2]
  so qkv matmuls produce M=128-packed outputs with lane-aligned copies.
- Heads 0/1 are row-packed BY HEAD in the score matmuls (no q/k replication);
  head 2 is row-packed across key-tile pairs with DMA-replicated q2/k2 halves.
- exp(scores) is split across ScalarE (true exp ACT) and VectorE (Schraudolph:
  i16 = s*128/ln2 + B computed by one tensor_scalar, bitcast as bf16) to break
  the ACT-engine bottleneck (~440us of the 660us baseline).
- Softmax denominator comes free from a ones-column appended to V (PSUM row 64
  of the PV accumulator); 1/S via reciprocal_approx_fast (the exact DVE
  reciprocal at 6 cyc/elem cost 80us + long waits in the baseline).
"""

import math
import os
import sys

import numpy as np

B, NSEQ, D = 4, 4096, 384
H, DH = 6, 64
HPC = 3  # heads per core
NCORES = 8
GCOLS = HPC * DH  # 192
SCALE = float(DH) ** -0.5

# Schraudolph exp in bf16-bitpattern domain: i16 = round(s*128/ln2 + B)
EXP_M = 128.0 / math.log(2.0)
EXP_B = 128.0 * 127.0 - 128.0 * (math.log2(1.0861) / 2.0) + 0.5

# Fraction of exp tiles handled by ScalarE (rest go to VectorE Schraudolph)
ACT_FRAC = 0.58
PV_DELAY = 14  # PV matmul entries the queue trails by (2 per score pair)


def _ensure_path():
    p = "/opt/trn_rl_repo"
    if os.path.isdir(p) and p not in sys.path:
        sys.path.insert(0, p)


def build_nc(n_seq=NSEQ):
    """Build the single-core Bass program (SPMD across 8 cores)."""
    _ensure_path()
    from contextlib import ExitStack

    import concourse.bacc as bacc
    import concourse.tile as tile
    from concourse import mybir

    f32 = mybir.dt.float32
    bf16 = mybir.dt.bfloat16
    i16 = mybir.dt.int16
    EXP = mybir.ActivationFunctionType.Exp
    MULT = mybir.AluOpType.mult
    ADD = mybir.AluOpType.add

    assert n_seq % 512 == 0
    nt = n_seq // 128
    nch = n_seq // 512

    nc = bacc.Bacc("TRN2", target_bir_lowering=False, debug=False)

    xT_d = nc.dram_tensor("xT", [D, n_seq], bf16, kind="ExternalInput")
    wT_d = nc.dram_tensor("wT", [128, 3 * 576], bf16, kind="ExternalInput")
    wpT_d = nc.dram_tensor("wpT", [64, 3 * D], bf16, kind="ExternalInput")
    y_d = nc.dram_tensor("yT", [D, n_seq], f32, kind="ExternalOutput")

    dbg = os.environ.get("KERNEL_DEBUG", "") == "1"
    if dbg:
        dbg_qk_d = nc.dram_tensor("dbg_qk", [128, 4, n_seq], bf16, kind="ExternalOutput")
        dbg_v_d = nc.dram_tensor("dbg_v", [128, (n_seq // 128) * 3 * 65], bf16, kind="ExternalOutput")
        dbg_e_d = nc.dram_tensor("dbg_e", [128, 4, 2, 512], bf16, kind="ExternalOutput")
        dbg_h_d = nc.dram_tensor("dbg_h", [64, 3, 512], bf16, kind="ExternalOutput")

    with tile.TileContext(nc) as tc, ExitStack() as ctx:
        constp = ctx.enter_context(tc.tile_pool(name="const", bufs=1))
        persist = ctx.enter_context(tc.tile_pool(name="persist", bufs=1))
        hpool = ctx.enter_context(tc.tile_pool(name="headsp", bufs=2))
        ypool = ctx.enter_context(tc.tile_pool(name="youtp", bufs=3))
        epool = ctx.enter_context(tc.tile_pool(name="epool", bufs=11))
        rcpool = ctx.enter_context(tc.tile_pool(name="rcp", bufs=2))

        ones = constp.tile([65, 64], f32)
        nc.vector.memset(ones, 1.0)

        # Transposed weights (host-prepped): wT[p, d3, col] = W_sel[col, 128*d3+p]
        wT = persist.tile([128, 3, 576], bf16)
        wpT = persist.tile([64, 3, D], bf16)
        # xT[p, d3, n] = x[n, 128*d3 + p] (host-prepped, bf16)
        xT = persist.tile([128, 3, n_seq], bf16)
        # q/k transposed layouts [c, n]:
        #   tq01/tk01: partitions 0:64 = head0, 64:128 = head1 (no replication)
        #   tq2/tk2: head2 replicated across both halves (via DMA)
        tq01 = persist.tile([128, n_seq], bf16)
        tk01 = persist.tile([128, n_seq], bf16)
        tq2 = persist.tile([128, n_seq], bf16)
        tk2 = persist.tile([128, n_seq], bf16)
        # V natural layout per key-tile, per head, ones column at col 64
        v_sb = persist.tile([128, nt, 3, 65], bf16)
        ones_nt = constp.tile([128, nt * 3], f32)
        nc.vector.memset(ones_nt, 1.0)
        nc.vector.tensor_copy(
            v_sb[:, :, :, 64:65].rearrange("p t h o -> p (t h o)"), ones_nt
        )

        nc.sync.dma_start(out=wT.rearrange("p a b -> p (a b)"), in_=wT_d[:, :])
        nc.sync.dma_start(out=wpT.rearrange("p a b -> p (a b)"), in_=wpT_d[:, :])
        # x loads chunk-major so qkv chunk 0 can start early
        for j in range(nch):
            for d3 in range(3):
                nc.sync.dma_start(
                    out=xT[:, d3, 512 * j : 512 * (j + 1)],
                    in_=xT_d[128 * d3 : 128 * (d3 + 1), 512 * j : 512 * (j + 1)],
                )

        # ---- qkv ----
        with (
            tc.tile_pool(name="pqk", bufs=4, space="PSUM") as pqk,
            tc.tile_pool(name="pvp", bufs=3, space="PSUM") as pvp,
        ):
            for j in range(nch):
                jsl = slice(512 * j, 512 * (j + 1))
                for c0, copies in (
                    (0, "q01"),
                    (128, "k01"),
                    (256, "q2k2"),
                ):
                    ps = pqk.tile([128, 512], f32, tag="pqk")
                    for d3 in range(3):
                        nc.tensor.matmul(
                            ps,
                            wT[:, d3, c0 : c0 + 128],
                            xT[:, d3, jsl],
                            start=(d3 == 0),
                            stop=(d3 == 2),
                        )
                    if copies == "q01":
                        nc.vector.tensor_copy(tq01[:, jsl], ps)
                    elif copies == "k01":
                        nc.vector.tensor_copy(tk01[:, jsl], ps)
                    else:
                        nc.scalar.copy(tq2[0:64, jsl], ps[0:64, :])
                        nc.scalar.copy(tk2[64:128, jsl], ps[64:128, :])
                for tt in range(4 * j, 4 * j + 4):
                    psv = pvp.tile([128, 192], f32, tag="pv")
                    for d3 in range(3):
                        nc.tensor.matmul(
                            psv,
                            xT[:, d3, 128 * tt : 128 * (tt + 1)],
                            wT[:, d3, 384:576],
                            start=(d3 == 0),
                            stop=(d3 == 2),
                        )
                    nc.scalar.copy(
                        v_sb[:, tt, :, 0:64],
                        psv[:, :].rearrange("p (h c) -> p h c", h=3),
                    )

        # replicate head2 q/k across the other partition half
        nc.sync.dma_start(out=tq2[64:128, :], in_=tq2[0:64, :])
        nc.sync.dma_start(out=tk2[0:64, :], in_=tk2[64:128, :])

        if dbg:
            for i, tqk in enumerate((tq01, tk01, tq2, tk2)):
                nc.sync.dma_start(out=dbg_qk_d[:, i, :], in_=tqk)
            nc.sync.dma_start(
                out=dbg_v_d[:, :], in_=v_sb.rearrange("p t h o -> p (t h o)")
            )

        # ---- attention + projection ----
        with (
            tc.tile_pool(name="psc", bufs=3, space="PSUM") as psc,
            tc.tile_pool(name="pop", bufs=2, space="PSUM") as pop,
        ):
            exp_state = [0.0]

            def exp_op(e_t, ps):
                # per-bank halves: each half's exp depends only on its own
                # score matmul and frees its PSUM bank independently
                exp_state[0] += ACT_FRAC
                if exp_state[0] >= 1.0:
                    exp_state[0] -= 1.0
                    for s in range(2):
                        nc.scalar.activation(e_t[:, s, :], ps[:, s, :], EXP)
                else:
                    for s in range(2):
                        nc.vector.tensor_scalar(
                            out=e_t[:, s, :].bitcast(i16),
                            in0=ps[:, s, :],
                            scalar1=EXP_M,
                            scalar2=EXP_B,
                            op0=MULT,
                            op1=ADD,
                        )

            pv_queue = []
            stops_emitted = set()  # tokens whose stop-matmul has been traced
            token_ctr = [0]

            def new_token():
                token_ctr[0] += 1
                return token_ctr[0]

            def flush_pv():
                po, vap, eap, st, sp = pv_queue.pop(0)
                nc.tensor.matmul(po, vap, eap, start=st, stop=(sp is not None))
                if sp is not None:
                    stops_emitted.add(sp)

            def pop_tail(pending):
                """Emit the next tail part iff its po's stop-MM is traced."""
                if not pending:
                    return
                po_id, part = pending[0]
                if po_id in stops_emitted:
                    pending.pop(0)
                    part()

            def drain_tail(pending):
                while pending:
                    po_id, part = pending[0]
                    while po_id not in stops_emitted:
                        flush_pv()
                    pending.pop(0)
                    part()

            def make_tail(j, h, po_t, tok, headsT):
                # The tail is split so (a) the PSUM accumulator po_t is freed
                # by its FIRST part (one ScalarE copy to SBUF) — the next
                # chunk's PV matmuls WAR-wait on that bank at the PE FIFO
                # head otherwise — and (b) the broadcast matmul is emitted a
                # few iterations later, after the reciprocal chain has
                # cleared the busy ACT/DVE queues.
                # normalize is split into two parts popped on consecutive
                # iterations so its engine ops spread across the ACT/DVE
                # queues instead of landing ahead of the exp ops that free
                # score-PSUM banks (the chunk-boundary PE stall).
                state = {}

                def norm_a():
                    rc = rcpool.tile([65, 2, 512], f32, tag="rc")
                    state["rc"] = rc
                    # reciprocal_approx_fast requires base partition 0 — copy
                    # the whole tile (denominator row 64 included) to SBUF and
                    # run the approx over all 65 rows (cost is free-dim-bound)
                    nc.scalar.copy(rc[:, 0, :], po_t[:, :])
                    nc.vector.reciprocal_approx_fast(rc[:, 1, :], rc[:, 0, :])

                def norm_b():
                    rc = state["rc"]
                    pb = psc.tile([64, 512], f32, tag="ps")
                    nc.tensor.matmul(pb, ones[64:65, :], rc[64:65, 1, :])
                    rb = rcpool.tile([64, 512], f32, tag="rb")
                    nc.scalar.copy(rb, pb)
                    nc.vector.tensor_mul(headsT[:, h, :], po_t[0:64, :], rb)

                def proj(e3):
                    def f():
                        pp = psc.tile([128, 512], f32, tag="ps")
                        for hh in range(3):
                            nc.tensor.matmul(
                                pp,
                                wpT[:, hh, 128 * e3 : 128 * (e3 + 1)],
                                headsT[:, hh, :],
                                start=(hh == 0),
                                stop=(hh == 2),
                            )
                        y_sb = ypool.tile([128, 512], f32, tag="y_sb")
                        nc.scalar.copy(y_sb, pp)
                        nc.sync.dma_start(
                            out=y_d[
                                128 * e3 : 128 * (e3 + 1), 512 * j : 512 * (j + 1)
                            ],
                            in_=y_sb,
                        )

                    return f

                parts = [(tok, norm_a), (tok, norm_b)]
                if h == 2:
                    parts += [(tok, proj(e3)) for e3 in range(3)]
                return parts

            pending_tail = []
            for j in range(nch):
                jsl = slice(512 * j, 512 * (j + 1))
                headsT = hpool.tile([64, 3, 512], bf16, tag="headsT")

                # phase A: heads 0/1 row-packed by head, one key-tile per iter
                po0 = pop.tile([65, 512], f32, tag="po")
                po1 = pop.tile([65, 512], f32, tag="po")
                tok0, tok1 = new_token(), new_token()
                for t in range(nt):
                    while len(pv_queue) > PV_DELAY:
                        flush_pv()
                    tsl = slice(128 * t, 128 * (t + 1))
                    ps = psc.tile([128, 2, 512], f32, tag="ps")
                    nc.tensor.matmul(ps[:, 0, :], tk01[0:64, tsl], tq01[0:64, jsl])
                    nc.tensor.matmul(
                        ps[:, 1, :], tk01[64:128, tsl], tq01[64:128, jsl]
                    )
                    e_t = epool.tile([128, 2, 512], bf16, tag="e_t")
                    exp_op(e_t, ps)
                    if dbg and j == 0 and t < 4:
                        nc.sync.dma_start(out=dbg_e_d[:, t, :, :], in_=e_t)
                    pv_queue.append(
                        (
                            po0,
                            v_sb[:, t, 0, :],
                            e_t[:, 0, :],
                            t == 0,
                            tok0 if t == nt - 1 else None,
                        )
                    )
                    pv_queue.append(
                        (
                            po1,
                            v_sb[:, t, 1, :],
                            e_t[:, 1, :],
                            t == 0,
                            tok1 if t == nt - 1 else None,
                        )
                    )
                    pop_tail(pending_tail)
                drain_tail(pending_tail)
                pending_tail = make_tail(j, 0, po0, tok0, headsT) + make_tail(
                    j, 1, po1, tok1, headsT
                )

                # phase B: head 2 row-packed across key-tile pairs
                po2 = pop.tile([65, 512], f32, tag="po")
                tok2 = new_token()
                for tp in range(nt // 2):
                    while len(pv_queue) > PV_DELAY:
                        flush_pv()
                    t0 = 2 * tp
                    ps = psc.tile([128, 2, 512], f32, tag="ps")
                    nc.tensor.matmul(
                        ps[:, 0, :],
                        tk2[0:64, 128 * t0 : 128 * (t0 + 1)],
                        tq2[0:64, jsl],
                    )
                    nc.tensor.matmul(
                        ps[:, 1, :],
                        tk2[64:128, 128 * (t0 + 1) : 128 * (t0 + 2)],
                        tq2[64:128, jsl],
                    )
                    e_t = epool.tile([128, 2, 512], bf16, tag="e_t")
                    exp_op(e_t, ps)
                    pv_queue.append(
                        (po2, v_sb[:, t0, 2, :], e_t[:, 0, :], t0 == 0, None)
                    )
                    pv_queue.append(
                        (
                            po2,
                            v_sb[:, t0 + 1, 2, :],
                            e_t[:, 1, :],
                            False,
                            tok2 if t0 + 1 == nt - 1 else None,
                        )
                    )
                    pop_tail(pending_tail)
                drain_tail(pending_tail)
                pending_tail = make_tail(j, 2, po2, tok2, headsT)

            while pv_queue:
                flush_pv()
            drain_tail(pending_tail)
            if dbg:
                # headsT of the LAST chunk (hpool tag rotation: last allocated)
                nc.sync.dma_start(out=dbg_h_d[:, :, :], in_=headsT)

    nc.compile()
    return nc


def shard_inputs(x, W_qkv, W_proj):
    """Full inputs -> per-core input maps (host-side transpose + pack)."""
    import ml_dtypes

    bf16 = ml_dtypes.bfloat16
    x = np.asarray(x, dtype=np.float32)
    W_qkv = np.asarray(W_qkv, dtype=np.float32)
    W_proj = np.asarray(W_proj, dtype=np.float32)
    d = W_qkv.shape[1]
    in_maps = []
    for c in range(NCORES):
        b, g = divmod(c, 2)
        sl = slice(GCOLS * g, GCOLS * (g + 1))
        q = W_qkv[0 * d :][sl] * SCALE  # [192, 384] (scale folded into q)
        k = W_qkv[1 * d :][sl]
        v = W_qkv[2 * d :][sl]
        w_sel = np.concatenate(
            [q[0:64], q[64:128], k[0:64], k[64:128], q[128:192], k[128:192], v],
            axis=0,
        )  # [576, 384]
        # wT[p, d3, col] = w_sel[col, 128*d3 + p]
        wT = np.ascontiguousarray(
            w_sel.T.reshape(3, 128, 576).transpose(1, 0, 2).reshape(128, 3 * 576)
        ).astype(bf16)
        wp_g = W_proj[:, sl]  # [384, 192]
        # wpT[p, h, e] = wp_g[e, 64h + p]
        wpT = np.ascontiguousarray(
            wp_g.T.reshape(3, 64, d).transpose(1, 0, 2).reshape(64, 3 * d)
        ).astype(bf16)
        xT = np.ascontiguousarray(x[b].T).astype(bf16)
        in_maps.append({"xT": xT, "wT": wT, "wpT": wpT})
    return in_maps


def combine_outputs(yTs, b_proj, n_seq=NSEQ):
    """Per-core yT partials -> full [B, N, D] output."""
    b_proj = np.asarray(b_proj, dtype=np.float32)
    y = np.empty((B, n_seq, D), dtype=np.float32)
    for b in range(B):
        y[b] = (yTs[2 * b] + yTs[2 * b + 1]).T + b_proj
    return y


_NC_CACHE = {}


def kernel(**inputs):
    _ensure_path()
    from concourse.bass_utils import run_bass_kernel_spmd

    x = np.asarray(inputs["x"], dtype=np.float32)
    W_qkv = np.asarray(inputs["W_qkv"], dtype=np.float32)
    W_proj = np.asarray(inputs["W_proj"], dtype=np.float32)
    b_proj = np.asarray(inputs["b_proj"], dtype=np.float32)

    n_seq = x.shape[1]
    if n_seq not in _NC_CACHE:
        _NC_CACHE[n_seq] = build_nc(n_seq)
    nc = _NC_CACHE[n_seq]

    in_maps = shard_inputs(x, W_qkv, W_proj)
    res = run_bass_kernel_spmd(nc, in_maps, core_ids=list(range(NCORES)))
    yTs = [r["yT"] for r in res.results]
    return combine_outputs(yTs, b_proj, n_seq)


if __name__ == "__main__":
    rng = np.random.default_rng(0)
    n = 512
    x = rng.standard_normal((B, n, D), dtype=np.float32)
    wq = (rng.standard_normal((3 * D, D), dtype=np.float32) / np.sqrt(D)).astype(
        np.float32
    )
    wp = (rng.standard_normal((D, D), dtype=np.float32) / np.sqrt(D)).astype(
        np.float32
    )
    bp = np.zeros(D, np.float32)
    out = kernel(x=x, W_qkv=wq, W_proj=wp, b_proj=bp)
    print(out.shape, out.dtype)


# revision 45
# speedup vs baseline: 1.0152x; 1.0092x over previous
"""Trainium2 Bass kernel for multi-head attention (B=4, N=4096, D=384, H=6).

Sharding: 8 cores = 4 batches x 2 head-groups (3 heads each).

Design notes:
- Host pre-transposes x (bf16) and weights (wT/wpT bf16), folding the 1/sqrt(dh)
  scale into the q rows of W_qkv. No PE transposes on device.
- W column order per group: [q_h0 q_h1 | k_h0 k_h1 | q_h2 k_h2 | v_h0 v_h1 v_h2]
  so qkv matmuls produce M=128-packed outputs with lane-aligned copies.
- Heads 0/1 are row-packed BY HEAD in the score matmuls (no q/k replication);
  head 2 is row-packed across key-tile pairs with DMA-replicated q2/k2 halves.
- exp(scores) is split across ScalarE (true exp ACT) and VectorE (Schraudolph:
  i16 = s*128/ln2 + B computed by one tensor_scalar, bitcast as bf16) to break
  the ACT-engine bottleneck (~440us of the 660us baseline).
- Softmax denominator comes free from a ones-column appended to V (PSUM row 64
  of the PV accumulator); 1/S via reciprocal_approx_fast (the exact DVE
  reciprocal at 6 cyc/elem cost 80us + long waits in the baseline).
"""

import math
import os
import sys

import numpy as np

B, NSEQ, D = 4, 4096, 384
H, DH = 6, 64
HPC = 3  # heads per core
NCORES = 8
GCOLS = HPC * DH  # 192
SCALE = float(DH) ** -0.5

# Schraudolph exp in bf16-bitpattern domain: i16 = round(s*128/ln2 + B)
EXP_M = 128.0 / math.log(2.0)
EXP_B = 128.0 * 127.0 - 128.0 * (math.log2(1.0861) / 2.0) + 0.5

# Fraction of exp tiles handled by ScalarE (rest go to VectorE Schraudolph)
ACT_FRAC = 0.58
PV_DELAY = 20  # PV matmul entries the queue trails by (2 per score pair)


def _ensure_path():
    p = "/opt/trn_rl_repo"
    if os.path.isdir(p) and p not in sys.path:
        sys.path.insert(0, p)


def build_nc(n_seq=NSEQ):
    """Build the single-core Bass program (SPMD across 8 cores)."""
    _ensure_path()
    from contextlib import ExitStack

    import concourse.bacc as bacc
    import concourse.tile as tile
    from concourse import mybir

    f32 = mybir.dt.float32
    bf16 = mybir.dt.bfloat16
    i16 = mybir.dt.int16
    EXP = mybir.ActivationFunctionType.Exp
    MULT = mybir.AluOpType.mult
    ADD = mybir.AluOpType.add

    assert n_seq % 512 == 0
    nt = n_seq // 128
    nch = n_seq // 512

    nc = bacc.Bacc("TRN2", target_bir_lowering=False, debug=False)

    xT_d = nc.dram_tensor("xT", [D, n_seq], bf16, kind="ExternalInput")
    wT_d = nc.dram_tensor("wT", [128, 3 * 576], bf16, kind="ExternalInput")
    wpT_d = nc.dram_tensor("wpT", [64, 3 * D], bf16, kind="ExternalInput")
    y_d = nc.dram_tensor("yT", [D, n_seq], f32, kind="ExternalOutput")

    dbg = os.environ.get("KERNEL_DEBUG", "") == "1"
    if dbg:
        dbg_qk_d = nc.dram_tensor("dbg_qk", [128, 4, n_seq], bf16, kind="ExternalOutput")
        dbg_v_d = nc.dram_tensor("dbg_v", [128, (n_seq // 128) * 3 * 65], bf16, kind="ExternalOutput")
        dbg_e_d = nc.dram_tensor("dbg_e", [128, 4, 2, 512], bf16, kind="ExternalOutput")
        dbg_h_d = nc.dram_tensor("dbg_h", [64, 3, 512], bf16, kind="ExternalOutput")

    with tile.TileContext(nc) as tc, ExitStack() as ctx:
        constp = ctx.enter_context(tc.tile_pool(name="const", bufs=1))
        persist = ctx.enter_context(tc.tile_pool(name="persist", bufs=1))
        hpool = ctx.enter_context(tc.tile_pool(name="headsp", bufs=2))
        ypool = ctx.enter_context(tc.tile_pool(name="youtp", bufs=3))
        epool = ctx.enter_context(tc.tile_pool(name="epool", bufs=14))
        rcpool = ctx.enter_context(tc.tile_pool(name="rcp", bufs=2))

        ones = constp.tile([65, 64], f32)
        nc.vector.memset(ones, 1.0)

        # Transposed weights (host-prepped): wT[p, d3, col] = W_sel[col, 128*d3+p]
        wT = persist.tile([128, 3, 576], bf16)
        wpT = persist.tile([64, 3, D], bf16)
        # xT[p, d3, n] = x[n, 128*d3 + p] (host-prepped, bf16)
        xT = persist.tile([128, 3, n_seq], bf16)
        # q/k transposed layouts [c, n]:
        #   tq01/tk01: partitions 0:64 = head0, 64:128 = head1 (no replication)
        #   tq2/tk2: head2 replicated across both halves (via DMA)
        tq01 = persist.tile([128, n_seq], bf16)
        tk01 = persist.tile([128, n_seq], bf16)
        tq2 = persist.tile([128, n_seq], bf16)
        tk2 = persist.tile([128, n_seq], bf16)
        # V natural layout per key-tile, per head, ones column at col 64
        v_sb = persist.tile([128, nt, 3, 65], bf16)
        ones_nt = constp.tile([128, nt * 3], f32)
        nc.vector.memset(ones_nt, 1.0)
        nc.vector.tensor_copy(
            v_sb[:, :, :, 64:65].rearrange("p t h o -> p (t h o)"), ones_nt
        )

        nc.sync.dma_start(out=wT.rearrange("p a b -> p (a b)"), in_=wT_d[:, :])
        nc.sync.dma_start(out=wpT.rearrange("p a b -> p (a b)"), in_=wpT_d[:, :])
        # x loads chunk-major so qkv chunk 0 can start early
        for j in range(nch):
            for d3 in range(3):
                nc.sync.dma_start(
                    out=xT[:, d3, 512 * j : 512 * (j + 1)],
                    in_=xT_d[128 * d3 : 128 * (d3 + 1), 512 * j : 512 * (j + 1)],
                )

        # ---- qkv ----
        with (
            tc.tile_pool(name="pqk", bufs=4, space="PSUM") as pqk,
            tc.tile_pool(name="pvp", bufs=3, space="PSUM") as pvp,
        ):
            for j in range(nch):
                jsl = slice(512 * j, 512 * (j + 1))
                for c0, copies in (
                    (0, "q01"),
                    (128, "k01"),
                    (256, "q2k2"),
                ):
                    ps = pqk.tile([128, 512], f32, tag="pqk")
                    for d3 in range(3):
                        nc.tensor.matmul(
                            ps,
                            wT[:, d3, c0 : c0 + 128],
                            xT[:, d3, jsl],
                            start=(d3 == 0),
                            stop=(d3 == 2),
                        )
                    if copies == "q01":
                        nc.vector.tensor_copy(tq01[:, jsl], ps)
                    elif copies == "k01":
                        nc.vector.tensor_copy(tk01[:, jsl], ps)
                    else:
                        nc.scalar.copy(tq2[0:64, jsl], ps[0:64, :])
                        nc.scalar.copy(tk2[64:128, jsl], ps[64:128, :])
                for tt in range(4 * j, 4 * j + 4):
                    psv = pvp.tile([128, 192], f32, tag="pv")
                    for d3 in range(3):
                        nc.tensor.matmul(
                            psv,
                            xT[:, d3, 128 * tt : 128 * (tt + 1)],
                            wT[:, d3, 384:576],
                            start=(d3 == 0),
                            stop=(d3 == 2),
                        )
                    nc.scalar.copy(
                        v_sb[:, tt, :, 0:64],
                        psv[:, :].rearrange("p (h c) -> p h c", h=3),
                    )

        # replicate head2 q/k across the other partition half
        nc.sync.dma_start(out=tq2[64:128, :], in_=tq2[0:64, :])
        nc.sync.dma_start(out=tk2[0:64, :], in_=tk2[64:128, :])

        if dbg:
            for i, tqk in enumerate((tq01, tk01, tq2, tk2)):
                nc.sync.dma_start(out=dbg_qk_d[:, i, :], in_=tqk)
            nc.sync.dma_start(
                out=dbg_v_d[:, :], in_=v_sb.rearrange("p t h o -> p (t h o)")
            )

        # ---- attention + projection ----
        with (
            tc.tile_pool(name="psc", bufs=3, space="PSUM") as psc,
            tc.tile_pool(name="pop", bufs=2, space="PSUM") as pop,
        ):
            exp_state = [0.0]

            def exp_op(e_t, ps):
                # per-bank halves: each half's exp depends only on its own
                # score matmul and frees its PSUM bank independently
                exp_state[0] += ACT_FRAC
                if exp_state[0] >= 1.0:
                    exp_state[0] -= 1.0
                    for s in range(2):
                        nc.scalar.activation(e_t[:, s, :], ps[:, s, :], EXP)
                else:
                    for s in range(2):
                        nc.vector.tensor_scalar(
                            out=e_t[:, s, :].bitcast(i16),
                            in0=ps[:, s, :],
                            scalar1=EXP_M,
                            scalar2=EXP_B,
                            op0=MULT,
                            op1=ADD,
                        )

            pv_queue = []
            stops_emitted = set()  # tokens whose stop-matmul has been traced
            token_ctr = [0]

            def new_token():
                token_ctr[0] += 1
                return token_ctr[0]

            def flush_pv():
                po, vap, eap, st, sp = pv_queue.pop(0)
                nc.tensor.matmul(po, vap, eap, start=st, stop=(sp is not None))
                if sp is not None:
                    stops_emitted.add(sp)

            def pop_tail(pending):
                """Emit the next tail part iff its po's stop-MM is traced."""
                if not pending:
                    return
                po_id, part = pending[0]
                if po_id in stops_emitted:
                    pending.pop(0)
                    part()

            def drain_tail(pending):
                while pending:
                    po_id, part = pending[0]
                    while po_id not in stops_emitted:
                        flush_pv()
                    pending.pop(0)
                    part()

            def make_tail(j, h, po_t, tok, headsT):
                # The tail is split so (a) the PSUM accumulator po_t is freed
                # by its FIRST part (one ScalarE copy to SBUF) — the next
                # chunk's PV matmuls WAR-wait on that bank at the PE FIFO
                # head otherwise — and (b) the broadcast matmul is emitted a
                # few iterations later, after the reciprocal chain has
                # cleared the busy ACT/DVE queues.
                # normalize is split into two parts popped on consecutive
                # iterations so its engine ops spread across the ACT/DVE
                # queues instead of landing ahead of the exp ops that free
                # score-PSUM banks (the chunk-boundary PE stall).
                state = {}

                def norm_a():
                    rc = rcpool.tile([65, 2, 512], f32, tag="rc")
                    state["rc"] = rc
                    # reciprocal_approx_fast requires base partition 0 — copy
                    # the whole tile (denominator row 64 included) to SBUF and
                    # run the approx over all 65 rows (cost is free-dim-bound)
                    nc.scalar.copy(rc[:, 0, :], po_t[:, :])
                    nc.vector.reciprocal_approx_fast(rc[:, 1, :], rc[:, 0, :])

                def norm_b():
                    rc = state["rc"]
                    pb = psc.tile([64, 512], f32, tag="ps")
                    nc.tensor.matmul(pb, ones[64:65, :], rc[64:65, 1, :])
                    rb = rcpool.tile([64, 512], f32, tag="rb")
                    nc.scalar.copy(rb, pb)
                    nc.vector.tensor_mul(headsT[:, h, :], po_t[0:64, :], rb)

                def proj(e3):
                    def f():
                        pp = psc.tile([128, 512], f32, tag="ps")
                        for hh in range(3):
                            nc.tensor.matmul(
                                pp,
                                wpT[:, hh, 128 * e3 : 128 * (e3 + 1)],
                                headsT[:, hh, :],
                                start=(hh == 0),
                                stop=(hh == 2),
                            )
                        y_sb = ypool.tile([128, 512], f32, tag="y_sb")
                        nc.scalar.copy(y_sb, pp)
                        nc.sync.dma_start(
                            out=y_d[
                                128 * e3 : 128 * (e3 + 1), 512 * j : 512 * (j + 1)
                            ],
                            in_=y_sb,
                        )

                    return f

                parts = [(tok, norm_a), (tok, norm_b)]
                if h == 2:
                    parts += [(tok, proj(e3)) for e3 in range(3)]
                return parts

            pending_tail = []
            for j in range(nch):
                jsl = slice(512 * j, 512 * (j + 1))
                headsT = hpool.tile([64, 3, 512], bf16, tag="headsT")

                # phase A: heads 0/1 row-packed by head, one key-tile per iter
                po0 = pop.tile([65, 512], f32, tag="po")
                po1 = pop.tile([65, 512], f32, tag="po")
                tok0, tok1 = new_token(), new_token()
                for t in range(nt):
                    while len(pv_queue) > PV_DELAY:
                        flush_pv()
                    tsl = slice(128 * t, 128 * (t + 1))
                    ps = psc.tile([128, 2, 512], f32, tag="ps")
                    nc.tensor.matmul(ps[:, 0, :], tk01[0:64, tsl], tq01[0:64, jsl])
                    nc.tensor.matmul(
                        ps[:, 1, :], tk01[64:128, tsl], tq01[64:128, jsl]
                    )
                    e_t = epool.tile([128, 2, 512], bf16, tag="e_t")
                    exp_op(e_t, ps)
                    if dbg and j == 0 and t < 4:
                        nc.sync.dma_start(out=dbg_e_d[:, t, :, :], in_=e_t)
                    pv_queue.append(
                        (
                            po0,
                            v_sb[:, t, 0, :],
                            e_t[:, 0, :],
                            t == 0,
                            tok0 if t == nt - 1 else None,
                        )
                    )
                    pv_queue.append(
                        (
                            po1,
                            v_sb[:, t, 1, :],
                            e_t[:, 1, :],
                            t == 0,
                            tok1 if t == nt - 1 else None,
                        )
                    )
                    pop_tail(pending_tail)
                drain_tail(pending_tail)
                pending_tail = make_tail(j, 0, po0, tok0, headsT) + make_tail(
                    j, 1, po1, tok1, headsT
                )

                # phase B: head 2 row-packed across key-tile pairs
                po2 = pop.tile([65, 512], f32, tag="po")
                tok2 = new_token()
                for tp in range(nt // 2):
                    while len(pv_queue) > PV_DELAY:
                        flush_pv()
                    t0 = 2 * tp
                    ps = psc.tile([128, 2, 512], f32, tag="ps")
                    nc.tensor.matmul(
                        ps[:, 0, :],
                        tk2[0:64, 128 * t0 : 128 * (t0 + 1)],
                        tq2[0:64, jsl],
                    )
                    nc.tensor.matmul(
                        ps[:, 1, :],
                        tk2[64:128, 128 * (t0 + 1) : 128 * (t0 + 2)],
                        tq2[64:128, jsl],
                    )
                    e_t = epool.tile([128, 2, 512], bf16, tag="e_t")
                    exp_op(e_t, ps)
                    pv_queue.append(
                        (po2, v_sb[:, t0, 2, :], e_t[:, 0, :], t0 == 0, None)
                    )
                    pv_queue.append(
                        (
                            po2,
                            v_sb[:, t0 + 1, 2, :],
                            e_t[:, 1, :],
                            False,
                            tok2 if t0 + 1 == nt - 1 else None,
                        )
                    )
                    pop_tail(pending_tail)
                drain_tail(pending_tail)
                pending_tail = make_tail(j, 2, po2, tok2, headsT)

            while pv_queue:
                flush_pv()
            drain_tail(pending_tail)
            if dbg:
                # headsT of the LAST chunk (hpool tag rotation: last allocated)
                nc.sync.dma_start(out=dbg_h_d[:, :, :], in_=headsT)

    nc.compile()
    return nc


def shard_inputs(x, W_qkv, W_proj):
    """Full inputs -> per-core input maps (host-side transpose + pack)."""
    import ml_dtypes

    bf16 = ml_dtypes.bfloat16
    x = np.asarray(x, dtype=np.float32)
    W_qkv = np.asarray(W_qkv, dtype=np.float32)
    W_proj = np.asarray(W_proj, dtype=np.float32)
    d = W_qkv.shape[1]
    in_maps = []
    for c in range(NCORES):
        b, g = divmod(c, 2)
        sl = slice(GCOLS * g, GCOLS * (g + 1))
        q = W_qkv[0 * d :][sl] * SCALE  # [192, 384] (scale folded into q)
        k = W_qkv[1 * d :][sl]
        v = W_qkv[2 * d :][sl]
        w_sel = np.concatenate(
            [q[0:64], q[64:128], k[0:64], k[64:128], q[128:192], k[128:192], v],
            axis=0,
        )  # [576, 384]
        # wT[p, d3, col] = w_sel[col, 128*d3 + p]
        wT = np.ascontiguousarray(
            w_sel.T.reshape(3, 128, 576).transpose(1, 0, 2).reshape(128, 3 * 576)
        ).astype(bf16)
        wp_g = W_proj[:, sl]  # [384, 192]
        # wpT[p, h, e] = wp_g[e, 64h + p]
        wpT = np.ascontiguousarray(
            wp_g.T.reshape(3, 64, d).transpose(1, 0, 2).reshape(64, 3 * d)
        ).astype(bf16)
        xT = np.ascontiguousarray(x[b].T).astype(bf16)
        in_maps.append({"xT": xT, "wT": wT, "wpT": wpT})
    return in_maps


def combine_outputs(yTs, b_proj, n_seq=NSEQ):
    """Per-core yT partials -> full [B, N, D] output."""
    b_proj = np.asarray(b_proj, dtype=np.float32)
    y = np.empty((B, n_seq, D), dtype=np.float32)
    for b in range(B):
        y[b] = (yTs[2 * b] + yTs[2 * b + 1]).T + b_proj
    return y


_NC_CACHE = {}


def kernel(**inputs):
    _ensure_path()
    from concourse.bass_utils import run_bass_kernel_spmd

    x = np.asarray(inputs["x"], dtype=np.float32)
    W_qkv = np.asarray(inputs["W_qkv"], dtype=np.float32)
    W_proj = np.asarray(inputs["W_proj"], dtype=np.float32)
    b_proj = np.asarray(inputs["b_proj"], dtype=np.float32)

    n_seq = x.shape[1]
    if n_seq not in _NC_CACHE:
        _NC_CACHE[n_seq] = build_nc(n_seq)
    nc = _NC_CACHE[n_seq]

    in_maps = shard_inputs(x, W_qkv, W_proj)
    res = run_bass_kernel_spmd(nc, in_maps, core_ids=list(range(NCORES)))
    yTs = [r["yT"] for r in res.results]
    return combine_outputs(yTs, b_proj, n_seq)


if __name__ == "__main__":
    rng = np.random.default_rng(0)
    n = 512
    x = rng.standard_normal((B, n, D), dtype=np.float32)
    wq = (rng.standard_normal((3 * D, D), dtype=np.float32) / np.sqrt(D)).astype(
        np.float32
    )
    wp = (rng.standard_normal((D, D), dtype=np.float32) / np.sqrt(D)).astype(
        np.float32
    )
    bp = np.zeros(D, np.float32)
    out = kernel(x=x, W_qkv=wq, W_proj=wp, b_proj=bp)
    print(out.shape, out.dtype)
